# revision 31
# baseline (speedup 1.0000x reference)
"""Fused decoder-layer kernel for one TRN2 chip (8 NeuronCores).

Problem (B=2, S=2048, DIM=1024, H=16, DH=64, DFF=2048):
    h1 = MHA(q=de_x, k=de_x, v=de_x, mask)   (shared per-head weights Wq/Wk/Wv)
    h2 = MHA(q=en_x, k=en_x, v=h1,  None)
    y  = relu(h2 @ W1 + b1) @ W2 + b2

Sharding: core c = 4*b + g  (b = batch, g = head-group of 4 heads; g also
indexes the S/4 slice of rows this core runs the FFN on).

Layout strategy (all matmul operands pre-transposed so every contraction has
its reduction dim on SBUF partitions; all matmul inputs bf16, PSUM f32):
  - host passes x^T [DIM, S] per batch; projections produce q^T/k^T [DH, S]
    and v [S, DH] directly.
  - scores are computed transposed ([s2, s1]) so P^T feeds the PV matmul as
    stationary weights with K = s2 on partitions.
  - a ones-column appended to v makes the PV matmul also produce softmax row
    sums (flash-attention style); normalization happens on the [DH, S] output.
  - FFN runs column-transposed (ff1^T, y^T) so b1/b2 are per-partition ACT
    biases and no transposes are ever needed. Host un-transposes y^T.
Cross-core:
  - attn2's V: instead of AllGathering h1, every core projects its LOCAL
    256-feature h1 block against the matching 256-row slice of the full-head
    Wv stack (wv_rs input), producing a partial v2 for ALL 16 heads; a 4-way
    bf16 ReduceScatter (chunked by head-group features) sums the partials and
    leaves each core exactly its own heads' v2 in [s, e] layout. Output
    bytes halve vs the AllGather, so the collective is ~26us cheaper and the
    PE cost is identical (same contraction, different factor order).
  - h2^T is exchanged with one 8-way AllToAll so each core ends up with its
    fixed S/4 column slice (SPMD program has no per-core offsets; 4-way
    AllToAll is rejected by the mesh check). Shards from the other batch land
    in known row blocks and are zeroed via a per-core row-mask input before
    the FFN contraction.
"""

import math

import numpy as np
import ml_dtypes

import concourse.bass as bass
import concourse.bacc as bacc
import concourse.mybir as mybir
import concourse.tile as tile
import concourse.bass_utils as bass_utils

B, S, DIM, H = 2, 2048, 1024, 16
DH = DIM // H            # 64
DFF = 2048
NEG = -1.0e9
N_CORES = 8
G = 4                    # cores per batch group == head-groups == s-groups
HPC = H // G             # heads per core = 4
SL = S // G              # FFN rows per core = 512
KC = DIM // 128          # 8 k-chunks of the model dim
NB = S // 128            # 16 key blocks
NT = S // 1024           # 2 query tiles of 1024
BF16 = mybir.dt.bfloat16
F32 = mybir.dt.float32
AF = mybir.ActivationFunctionType

_CACHE: dict = {}


def _mask_plan(mask: np.ndarray):
    """Classify each [1024 x 128] (s1, s2) block: 'N' no-op, 'M' apply, 'S' skip."""
    plan = []
    for t in range(NT):
        row = []
        for blk in range(NB):
            sub = mask[t * 1024:(t + 1) * 1024, blk * 128:(blk + 1) * 128]
            # NOTE: an 'S' (skip-block) fast path deadlocked on hardware;
            # fully-masked blocks are handled as 'M' (exp underflows to 0).
            row.append('N' if not sub.any() else 'M')
        plan.append(tuple(row))
    return tuple(plan)


def _build(plan):
    has_mask = any(c == 'M' for row in plan for c in row)
    nc = bacc.Bacc("TRN2", target_bir_lowering=False, debug=False,
                   num_devices=N_CORES)

    de_xT = nc.dram_tensor("de_xT", [DIM, S], BF16, kind="ExternalInput")
    en_xT = nc.dram_tensor("en_xT", [DIM, S], BF16, kind="ExternalInput")
    wq = nc.dram_tensor("wq", [DIM, HPC * DH], BF16, kind="ExternalInput")
    wk = nc.dram_tensor("wk", [DIM, HPC * DH], BF16, kind="ExternalInput")
    wv = nc.dram_tensor("wv", [DIM, HPC * DH], BF16, kind="ExternalInput")
    wv_rs = nc.dram_tensor("wv_rs", [128, 2, H * DH], BF16,
                           kind="ExternalInput")
    w1 = nc.dram_tensor("w1", [DIM, DFF], BF16, kind="ExternalInput")
    w2 = nc.dram_tensor("w2", [DFF, DIM], BF16, kind="ExternalInput")
    b1t = nc.dram_tensor("b1t", [128, DFF // 128], F32, kind="ExternalInput")
    b2t = nc.dram_tensor("b2t", [128, DIM // 128], F32, kind="ExternalInput")
    zmask = nc.dram_tensor("zmask", [128, 2 * KC], F32, kind="ExternalInput")
    maskT = None
    if has_mask:
        maskT = nc.dram_tensor("maskT", [S, S], BF16, kind="ExternalInput")
    yT = nc.dram_tensor("yT", [DIM, SL], F32, kind="ExternalOutput")

    with tile.TileContext(nc) as tc:
        _trace(nc, tc, plan, de_xT, en_xT, wq, wk, wv, wv_rs, w1, w2,
               b1t, b2t, zmask, maskT, yT)
    nc.compile()
    return nc, has_mask


def _trace(nc, tc, plan, de_xT, en_xT, wq, wk, wv, wv_rs, w1, w2, b1t, b2t,
           zmask, maskT, yT):
    # Unified allocation stack: everything (pools and single tiles) must be
    # released in strict LIFO order before TileContext exits.
    stack = nc._tile_stack = []   # [release_fn or None(done)]

    def _push(release_fn):
        ent = {"f": release_fn}
        stack.append(ent)
        def rel():
            assert ent["f"] is not None, "double release"
            ent["f"](); ent["f"] = None
        return rel

    def release_rest():
        for ent in reversed(stack):
            if ent["f"] is not None:
                ent["f"](); ent["f"] = None

    noplan = tuple(tuple('N' for _ in range(NB)) for _ in range(NT))

    # ---- pools ----------------------------------------------------------
    def pool(**kw):
        cm = tc.tile_pool(**kw)
        p = cm.__enter__()
        _push(lambda: cm.__exit__(None, None, None))
        return p

    def single(shape, dtype, name):
        t_, f_ = tc.tile(shape, dtype, name=name)
        return t_, _push(f_)

    ps_big = pool(name="ps_big", bufs=2, space="PSUM")
    ps_hext = pool(name="ps_hext", bufs=2, space="PSUM")
    pt_pool = pool(name="pt", bufs=6)
    rc_pool = pool(name="rc", bufs=2)
    bc_pool = pool(name="bc", bufs=2)
    q_pool = pool(name="qp", bufs=2)
    k_pool = pool(name="kp", bufs=2)
    v_pool = pool(name="vp", bufs=1)
    sh_pool = pool(name="shp", bufs=2)
    y_pool = pool(name="yp", bufs=2)
    dram = pool(name="dram", bufs=1, space="DRAM")

    # ---- persistent tiles (stack order: frees must pop LIFO) ------------
    wq_sb, _ = single([128, KC, HPC * DH], BF16, "wqsb")
    wk_sb, _ = single([128, KC, HPC * DH], BF16, "wksb")
    wv_sb, _ = single([128, KC, HPC * DH], BF16, "wvsb")
    # en allocated BELOW de on the stack: de is freed first (after attn1),
    # en later (after attn2 q/k projections).
    en_sb, en_free = [], []
    for kc in range(KC):
        t_, f_ = single([128, S], BF16, f"en{kc}")
        en_sb.append(t_); en_free.append(f_)
    de_sb, de_free = [], []
    for kc in range(KC):
        t_, f_ = single([128, S], BF16, f"de{kc}")
        de_sb.append(t_); de_free.append(f_)
    # freed right after the attn1 partial-v2 projections (LIFO: above de/en)
    wvrs_sb, wvrs_free = single([128, 2, H * DH], BF16, "wvrssb")
    # DMA trace order: attn1's operands first
    nc.sync.dma_start(wq_sb[:], wq.rearrange("(a p) c -> p a c", p=128))
    nc.sync.dma_start(wk_sb[:], wk.rearrange("(a p) c -> p a c", p=128))
    for kc in range(KC):
        nc.sync.dma_start(de_sb[kc][:], de_xT[kc * 128:(kc + 1) * 128, :])
    nc.sync.dma_start(wv_sb[:], wv.rearrange("(a p) c -> p a c", p=128))
    nc.sync.dma_start(wvrs_sb[:], wv_rs[:])
    for kc in range(KC):
        nc.sync.dma_start(en_sb[kc][:], en_xT[kc * 128:(kc + 1) * 128, :])

    # split collective bounce buffers: partial-v2 ReduceScatter per
    # query-tile half, A2A per head-pair half — each half fires as soon as
    # its producer loop is done, so the wire time hides under the remaining
    # attention compute.
    rs_in = [dram.tile([G * 1024, 256], BF16, name=f"rsi{t}")
             for t in range(NT)]
    rs_out = [dram.tile([1024, 256], BF16, name=f"rso{t}")
              for t in range(NT)]
    cc2_in = [dram.tile([2 * G * 128, SL], BF16, name=f"cc2i{p}")
              for p in range(2)]
    cc2_out = [dram.tile([2 * G * 128, SL], BF16, name=f"cc2o{p}")
               for p in range(2)]

    # ---- helpers --------------------------------------------------------
    def project_qk_pair(x_sb, w_sb, pool, pair):
        """q^T (or k^T) for one head pair as a [128, S] bf16 tile."""
        qt = pool.tile([128, S], BF16, tag="qk", name=f"qk{pair}")
        for st in range(NT):
            ps = ps_big.tile([128, 1024], F32, tag="ps", name="pjps")
            for kc in range(KC):
                for nn in (0, 512):
                    nc.tensor.matmul(
                        ps[:, nn:nn + 512],
                        w_sb[:, kc, pair * 128:(pair + 1) * 128],
                        x_sb[kc][:, st * 1024 + nn:st * 1024 + nn + 512],
                        start=(kc == 0), stop=(kc == KC - 1))
            nc.vector.tensor_copy(qt[:, st * 1024:(st + 1) * 1024], ps[:])
        return qt

    def project_v(src_sb, v_all, blk_lo, blk_hi):
        """v for 4 heads + ones column into v_all[128, NB*HPC*65] (bf16)."""
        for blk in range(blk_lo, blk_hi):
            ps = ps_big.tile([128, 1024], F32, tag="ps", name="vps")
            for kc in range(KC):
                nc.tensor.matmul(
                    ps[:, 0:HPC * DH],
                    src_sb[kc][:, blk * 128:(blk + 1) * 128],
                    wv_sb[:, kc, :],
                    start=(kc == 0), stop=(kc == KC - 1))
            for h in range(HPC):
                nc.vector.tensor_copy(
                    v_all[:, (blk * HPC + h) * 65:(blk * HPC + h) * 65 + 64],
                    ps[:, h * DH:(h + 1) * DH])

    def new_v_all():
        v_all = v_pool.tile([128, NB * HPC * 65], BF16, tag="v", name="vall")
        v3 = v_all[:].rearrange("p (b c) -> p b c", c=65)
        nc.vector.memset(v3[:, :, 64:65], 1.0)
        return v_all

    def attn_unit(q_pairs, k_pairs, v_all, aplan, mask_tiles_in, sh, h, t):
        """scores -> exp -> PV -> normalize for one (head, query-tile)."""
        pair, off = h // 2, (h % 2) * 64
        hext = ps_hext.tile([65, 1024], F32, name="hext")
        for blk in range(NB):
            sc = ps_big.tile([128, 1024], F32, tag="ps", name="scps")
            for nn in (0, 512):
                nc.tensor.matmul(
                    sc[:, nn:nn + 512],
                    k_pairs[pair][off:off + 64, blk * 128:(blk + 1) * 128],
                    q_pairs[pair][off:off + 64,
                                  t * 1024 + nn:t * 1024 + nn + 512],
                    start=True, stop=True)
            if aplan[t][blk] == 'M':
                nc.vector.tensor_add(sc[:], sc[:], mask_tiles_in[(t, blk)][:])
            pt = pt_pool.tile([128, 1024], BF16, name="pt")
            nc.scalar.activation(pt[:], sc[:], AF.Exp)
            vs = v_all[:, (blk * HPC + h) * 65:(blk * HPC + h) * 65 + 65]
            for nn in (0, 512):
                nc.tensor.matmul(
                    hext[:, nn:nn + 512], vs, pt[:, nn:nn + 512],
                    start=(blk == 0), stop=(blk == NB - 1))
        recip = rc_pool.tile([1, 1024], F32, name="recip")
        nc.vector.reciprocal(recip[:], hext[64:65, :])
        rbc = bc_pool.tile([64, 1024], F32, name="rbc")
        nc.gpsimd.partition_broadcast(rbc[:], recip[0:1, :])
        nc.vector.tensor_mul(
            sh[pair][off:off + 64, t * 1024:(t + 1) * 1024],
            hext[0:64, :], rbc[:])

    def new_sh():
        return [sh_pool.tile([128, S], BF16, tag="sh", name=f"sh{p}")
                for p in range(2)]

    # ---- attention 1 (self-attn on de_x, mask) --------------------------
    # pair-0 q/k and v project first so the exp pipeline (ACT) starts as
    # early as possible; pair-1 projections slot in behind the first units.
    q1 = [None, None]
    k1 = [None, None]
    q1[0] = project_qk_pair(de_sb, wq_sb, q_pool, 0)
    k1[0] = project_qk_pair(de_sb, wk_sb, k_pool, 0)
    v1 = new_v_all()
    project_v(de_sb, v1, 0, NB)
    mask_tiles, mask_free = {}, []
    for t in range(NT):
        for blk in range(NB):
            if plan[t][blk] == 'M':
                mt, fm = single([128, 1024], BF16, f"mk{t}_{blk}")
                nc.sync.dma_start(
                    mt[:], maskT[blk * 128:(blk + 1) * 128,
                                 t * 1024:(t + 1) * 1024])
                mask_tiles[(t, blk)] = mt
                mask_free.append(fm)
    # t-major so each query-tile half's partial-v2 ReduceScatter runs while
    # the other half is still computing. Chunk c' of rs_in is the feature
    # block owned by group peer c' — identical slicing on every core, so the
    # SPMD program stays offset-free.
    h1sh = new_sh()
    for t in range(NT):
        for h in range(HPC):
            if t == 0 and h == 2:
                q1[1] = project_qk_pair(de_sb, wq_sb, q_pool, 1)
                k1[1] = project_qk_pair(de_sb, wk_sb, k_pool, 1)
            attn_unit(q1, k1, v1, plan, mask_tiles, h1sh, h, t)
        for sblk in range(NB // NT):
            ps = ps_big.tile([128, 1024], F32, tag="ps", name="rsps")
            col = t * 1024 + sblk * 128
            for kc2 in range(2):
                for nn in (0, 512):
                    nc.tensor.matmul(
                        ps[:, nn:nn + 512],
                        h1sh[kc2][:, col:col + 128],
                        wvrs_sb[:, kc2, nn:nn + 512],
                        start=(kc2 == 0), stop=(kc2 == 1))
            pp = pt_pool.tile([128, 1024], BF16, name="pp")
            nc.vector.tensor_copy(pp[:], ps[:])
            for cp in range(G):
                nc.sync.dma_start(
                    rs_in[t][cp * 1024 + sblk * 128:
                             cp * 1024 + (sblk + 1) * 128, :],
                    pp[:, cp * 256:(cp + 1) * 256])
        nc.gpsimd.collective_compute(
            "ReduceScatter", mybir.AluOpType.add,
            replica_groups=[[0, 1, 2, 3], [4, 5, 6, 7]],
            ins=[rs_in[t].opt()], outs=[rs_out[t].opt()])
    for f in reversed(mask_free):
        f()
    wvrs_free()
    for f in reversed(de_free):
        f()

    # ---- attention 2 (q,k from en_x; v from reduce-scattered v2) --------
    q2 = [None, None]
    k2 = [None, None]
    q2[0] = project_qk_pair(en_sb, wq_sb, q_pool, 0)
    k2[0] = project_qk_pair(en_sb, wk_sb, k_pool, 0)
    q2[1] = project_qk_pair(en_sb, wq_sb, q_pool, 1)
    k2[1] = project_qk_pair(en_sb, wk_sb, k_pool, 1)
    v2 = new_v_all()
    for t in range(NT):
        vrs, vrs_free = single([128, NB // NT, 256], BF16, f"vrs{t}")
        nc.sync.dma_start(vrs[:], rs_out[t].rearrange("(a p) c -> p a c",
                                                      p=128))
        for sblk in range(NB // NT):
            blk = t * (NB // NT) + sblk
            for h in range(HPC):
                nc.vector.tensor_copy(
                    v2[:, (blk * HPC + h) * 65:(blk * HPC + h) * 65 + 64],
                    vrs[:, sblk, h * DH:(h + 1) * DH])
        vrs_free()
    for f in reversed(en_free):
        f()

    # FFN weights prefetch during attention 2
    w1_sb, w1_free = [], []
    for kc in range(KC):
        t_, f_ = single([128, DFF], BF16, f"w1_{kc}")
        nc.sync.dma_start(t_[:], w1[kc * 128:(kc + 1) * 128, :])
        w1_sb.append(t_); w1_free.append(f_)
    w2_sb, w2_free = [], []
    for dc in range(DFF // 128):
        t_, f_ = single([128, DIM], BF16, f"w2_{dc}")
        nc.sync.dma_start(t_[:], w2[dc * 128:(dc + 1) * 128, :])
        w2_sb.append(t_); w2_free.append(f_)
    b1_sb, _ = single([128, DFF // 128], F32, "b1sb")
    b2_sb, _ = single([128, DIM // 128], F32, "b2sb")
    zm_sb, _ = single([128, 2 * KC], F32, "zmsb")
    nc.sync.dma_start(b1_sb[:], b1t[:])
    nc.sync.dma_start(b2_sb[:], b2t[:])
    nc.sync.dma_start(zm_sb[:], zmask[:])

    # pair-major so each head-pair half of h2^T AllToAlls while the other
    # pair is still computing
    h2sh = new_sh()
    for pair in range(2):
        for hh in range(2):
            for t in range(NT):
                attn_unit(q2, k2, v2, noplan, {}, h2sh, pair * 2 + hh, t)
        for j in range(2 * G):
            nc.sync.dma_start(
                cc2_in[pair][j * 128:(j + 1) * 128, :],
                h2sh[pair][:, (j % G) * SL:(j % G + 1) * SL])
        nc.gpsimd.collective_compute(
            "AllToAll", mybir.AluOpType.bypass,
            replica_groups=[[0, 1, 2, 3, 4, 5, 6, 7]],
            ins=[cc2_in[pair].opt()], outs=[cc2_out[pair].opt()])

    # ---- FFN on own S/4 rows -------------------------------------------
    # The A2A delivers each feature row twice (once per batch group); zero the
    # foreign-batch copy via the zmask input, then fold the two copies
    # together so the FFN contraction only runs over the real 1024 rows.
    h2_sb, h2_free = [], []     # [pair][j], j in 0..G
    for pair in range(2):
        row = []
        for i in range(2 * G):
            t_, f_ = single([128, SL], BF16, f"h2_{pair}_{i}")
            nc.sync.dma_start(t_[:], cc2_out[pair][i * 128:(i + 1) * 128, :])
            nc.vector.tensor_scalar_mul(
                t_[:], t_[:], zm_sb[:, pair * 2 * G + i:pair * 2 * G + i + 1])
            row.append(t_); h2_free.append(f_)
        for j in range(G):
            nc.vector.tensor_add(row[j][:], row[j][:], row[j + G][:])
        h2_sb.append(row[:G])

    # pass A: pair-0 partial sums land in SBUF f32 while A2A-b is in flight
    fp_sb, fp_free = [], []
    for dffb in range(DFF // 128):
        ps = ps_big.tile([128, 1024], F32, tag="ps", name="fAps")
        for i in range(G):
            nc.tensor.matmul(
                ps[:, 0:SL],
                w1_sb[2 * i][:, dffb * 128:(dffb + 1) * 128],
                h2_sb[0][i][:],
                start=(i == 0), stop=(i == G - 1))
        t_, f_ = single([128, SL], BF16, f"fp_{dffb}")
        nc.vector.tensor_copy(t_[:], ps[:, 0:SL])
        fp_sb.append(t_); fp_free.append(f_)
    # pass B: pair-1 contribution + pass-A partial, relu, bias
    ff1_sb, ff1_free = [], []
    for dffb in range(DFF // 128):
        ps = ps_big.tile([128, 1024], F32, tag="ps", name="fBps")
        for i in range(G):
            nc.tensor.matmul(
                ps[:, 0:SL],
                w1_sb[2 * i + 1][:, dffb * 128:(dffb + 1) * 128],
                h2_sb[1][i][:],
                start=(i == 0), stop=(i == G - 1))
        nc.vector.tensor_add(ps[:, 0:SL], ps[:, 0:SL], fp_sb[dffb][:])
        t_, f_ = single([128, SL], BF16, f"ff1_{dffb}")
        nc.scalar.activation(t_[:], ps[:, 0:SL], AF.Relu,
                             bias=b1_sb[:, dffb:dffb + 1])
        ff1_sb.append(t_); ff1_free.append(f_)
    for dimb in range(DIM // 128):
        ps = ps_big.tile([128, 1024], F32, tag="ps", name="yps")
        for dc in range(DFF // 128):
            nc.tensor.matmul(
                ps[:, 0:SL],
                w2_sb[dc][:, dimb * 128:(dimb + 1) * 128],
                ff1_sb[dc][:],
                start=(dc == 0), stop=(dc == DFF // 128 - 1))
        ysb = y_pool.tile([128, SL], F32, tag="y", name="ysb")
        nc.vector.tensor_scalar_add(ysb[:], ps[:, 0:SL],
                                    b2_sb[:, dimb:dimb + 1])
        nc.sync.dma_start(yT[dimb * 128:(dimb + 1) * 128, :], ysb[:])

    release_rest()


def _prep_inputs(de_x, en_x, mask, Wq, Wk, Wv, W1, b1, W2, b2, has_mask):
    bf = ml_dtypes.bfloat16
    scale = 1.0 / math.sqrt(DH)
    in_maps = []
    deT = [np.ascontiguousarray(de_x[b].T).astype(bf) for b in range(B)]
    enT = [np.ascontiguousarray(en_x[b].T).astype(bf) for b in range(B)]
    wv_flat = np.transpose(Wv, (1, 0, 2)).reshape(DIM, H * DH)
    w1b = W1.astype(bf)
    w2b = W2.astype(bf)
    b1t = np.ascontiguousarray(b1.reshape(DFF // 128, 128).T).astype(np.float32)
    b2t = np.ascontiguousarray(b2.reshape(DIM // 128, 128).T).astype(np.float32)
    mT = None
    if has_mask:
        mT = np.ascontiguousarray(mask.T * np.float32(NEG)).astype(bf)
    for c in range(N_CORES):
        b, g = divmod(c, G)
        hs = slice(g * HPC, (g + 1) * HPC)
        m = {
            "de_xT": deT[b],
            "en_xT": enT[b],
            "wq": np.ascontiguousarray(
                np.transpose(Wq[hs] * scale, (1, 0, 2)).reshape(DIM, HPC * DH)
            ).astype(bf),
            "wk": np.ascontiguousarray(
                np.transpose(Wk[hs], (1, 0, 2)).reshape(DIM, HPC * DH)).astype(bf),
            "wv": np.ascontiguousarray(
                np.transpose(Wv[hs], (1, 0, 2)).reshape(DIM, HPC * DH)).astype(bf),
            "wv_rs": np.ascontiguousarray(
                wv_flat[g * 256:(g + 1) * 256]
                .reshape(2, 128, H * DH).transpose(1, 0, 2)).astype(bf),
            "w1": w1b, "w2": w2b, "b1t": b1t, "b2t": b2t,
        }
        zm = np.zeros((128, 2 * KC), np.float32)
        for pair in range(2):
            for i in range(2 * G):
                if i // G == b:
                    zm[:, pair * 2 * G + i] = 1.0
        m["zmask"] = zm
        if has_mask:
            m["maskT"] = mT
        in_maps.append(m)
    return in_maps


def get_program(mask):
    plan = _mask_plan(np.asarray(mask))
    if plan not in _CACHE:
        _CACHE[plan] = _build(plan)
    return _CACHE[plan]


_RUNNERS: dict = {}


def _fast_runner(nc):
    """Build (once) a cached jitted SPMD executor for this program.

    run_bass_kernel_spmd re-creates and re-traces its jax.jit closure on
    every call; caching the jitted shard_map shaves seconds of dispatch
    overhead off warm calls. Mirrors bass2jax.run_bass_via_pjrt.
    """
    import jax
    from jax.sharding import Mesh, PartitionSpec
    try:
        from jax.experimental.shard_map import shard_map
    except ImportError:
        from jax.shard_map import shard_map
    import concourse.mybir as _mb
    from concourse import bass2jax as b2j

    b2j.install_neuronx_cc_hook()
    partition_name = (nc.partition_id_tensor.name
                      if nc.partition_id_tensor else None)
    in_names, out_names, out_avals = [], [], []
    for alloc in nc.m.functions[0].allocations:
        if not isinstance(alloc, _mb.MemoryLocationSet):
            continue
        name = alloc.memorylocations[0].name
        if alloc.kind == "ExternalInput":
            if name != partition_name:
                in_names.append(name)
        elif alloc.kind == "ExternalOutput":
            out_names.append(name)
            out_avals.append(jax.core.ShapedArray(
                tuple(alloc.tensor_shape), _mb.dt.np(alloc.dtype)))
    n_params = len(in_names)
    n_outs = len(out_avals)
    all_names = in_names + out_names + ([partition_name] if partition_name else [])
    donate = tuple(range(n_params, n_params + n_outs))

    def _body(*args):
        operands = list(args)
        if partition_name is not None:
            operands.append(b2j.partition_id_tensor())
        return tuple(b2j._bass_exec_p.bind(
            *operands,
            out_avals=tuple(out_avals),
            in_names=tuple(all_names),
            out_names=tuple(out_names),
            lowering_input_output_aliases=(),
            sim_require_finite=True,
            sim_require_nnan=True,
            nc=nc,
        ))

    devices = jax.devices()[:N_CORES]
    mesh = Mesh(np.asarray(devices), ("core",))
    in_specs = (PartitionSpec("core"),) * (n_params + n_outs)
    out_specs = (PartitionSpec("core"),) * n_outs
    sharded = jax.jit(
        shard_map(_body, mesh=mesh, in_specs=in_specs, out_specs=out_specs,
                  check_rep=False),
        donate_argnums=donate, keep_unused=True)

    def runner(in_maps):
        concat_in = [np.concatenate([in_maps[c][nm] for c in range(N_CORES)],
                                    axis=0) for nm in in_names]
        zeros = [np.zeros((N_CORES * a.shape[0], *a.shape[1:]), a.dtype)
                 for a in out_avals]
        out_arrs = sharded(*concat_in, *zeros)
        return [
            {nm: np.asarray(out_arrs[i]).reshape(N_CORES, *out_avals[i].shape)[c]
             for i, nm in enumerate(out_names)}
            for c in range(N_CORES)
        ]

    return runner


def run(inputs, want_results=False, **run_kwargs):
    nc, has_mask = get_program(inputs["mask"])
    in_maps = _prep_inputs(
        inputs["de_x"], inputs["en_x"], inputs["mask"],
        inputs["Wq"], inputs["Wk"], inputs["Wv"],
        inputs["W1"], inputs["b1"], inputs["W2"], inputs["b2"], has_mask)
    results = None
    res = None
    if not run_kwargs:
        try:
            key = id(nc)
            if key not in _RUNNERS:
                _RUNNERS[key] = _fast_runner(nc)
            results = _RUNNERS[key](in_maps)
        except Exception:
            results = None
    if results is None:
        res = bass_utils.run_bass_kernel_spmd(
            nc, in_maps, core_ids=list(range(N_CORES)), **run_kwargs)
        results = res.results
    y = np.empty((B, S, DIM), np.float32)
    for c in range(N_CORES):
        b, g = divmod(c, G)
        y[b, g * SL:(g + 1) * SL, :] = results[c]["yT"].T
    return (y, res) if want_results else y


def kernel(**inputs) -> np.ndarray:
    return run({k: np.asarray(v) for k, v in inputs.items()})



# revision 37
# speedup vs baseline: 1.0069x; 1.0069x over previous
"""Fused decoder-layer kernel for one TRN2 chip (8 NeuronCores).

Problem (B=2, S=2048, DIM=1024, H=16, DH=64, DFF=2048):
    h1 = MHA(q=de_x, k=de_x, v=de_x, mask)   (shared per-head weights Wq/Wk/Wv)
    h2 = MHA(q=en_x, k=en_x, v=h1,  None)
    y  = relu(h2 @ W1 + b1) @ W2 + b2

Sharding: core c = 4*b + g  (b = batch, g = head-group of 4 heads; g also
indexes the S/4 slice of rows this core runs the FFN on).

Layout strategy (all matmul operands pre-transposed so every contraction has
its reduction dim on SBUF partitions; all matmul inputs bf16, PSUM f32):
  - host passes x^T [DIM, S] per batch; projections produce q^T/k^T [DH, S]
    and v [S, DH] directly.
  - scores are computed transposed ([s2, s1]) so P^T feeds the PV matmul as
    stationary weights with K = s2 on partitions.
  - a ones-column appended to v makes the PV matmul also produce softmax row
    sums (flash-attention style); normalization happens on the [DH, S] output.
  - FFN runs column-transposed (ff1^T, y^T) so b1/b2 are per-partition ACT
    biases and no transposes are ever needed. Host un-transposes y^T.
Cross-core:
  - attn2's V: instead of AllGathering h1, every core projects its LOCAL
    256-feature h1 block against the matching 256-row slice of the full-head
    Wv stack (wv_rs input), producing a partial v2 for ALL 16 heads; a 4-way
    bf16 ReduceScatter (chunked by head-group features) sums the partials and
    leaves each core exactly its own heads' v2 in [s, e] layout. Output
    bytes halve vs the AllGather, so the collective is ~26us cheaper and the
    PE cost is identical (same contraction, different factor order).
  - h2^T is exchanged with one 8-way AllToAll so each core ends up with its
    fixed S/4 column slice (SPMD program has no per-core offsets; 4-way
    AllToAll is rejected by the mesh check). Shards from the other batch land
    in known row blocks and are zeroed via a per-core row-mask input before
    the FFN contraction.
"""

import math

import numpy as np
import ml_dtypes

import concourse.bass as bass
import concourse.bacc as bacc
import concourse.mybir as mybir
import concourse.tile as tile
import concourse.bass_utils as bass_utils
from concourse import masks

B, S, DIM, H = 2, 2048, 1024, 16
DH = DIM // H            # 64
DFF = 2048
NEG = -1.0e9
N_CORES = 8
G = 4                    # cores per batch group == head-groups == s-groups
HPC = H // G             # heads per core = 4
SL = S // G              # FFN rows per core = 512
KC = DIM // 128          # 8 k-chunks of the model dim
NB = S // 128            # 16 key blocks
NT = S // 1024           # 2 query tiles of 1024
BF16 = mybir.dt.bfloat16
F32 = mybir.dt.float32
AF = mybir.ActivationFunctionType

_CACHE: dict = {}


def _mask_plan(mask: np.ndarray):
    """Classify each [1024 x 128] (s1, s2) block: 'N' no-op, 'M' apply, 'S' skip."""
    plan = []
    for t in range(NT):
        row = []
        for blk in range(NB):
            sub = mask[t * 1024:(t + 1) * 1024, blk * 128:(blk + 1) * 128]
            # NOTE: an 'S' (skip-block) fast path deadlocked on hardware;
            # fully-masked blocks are handled as 'M' (exp underflows to 0).
            row.append('N' if not sub.any() else 'M')
        plan.append(tuple(row))
    return tuple(plan)


def _build(plan):
    has_mask = any(c == 'M' for row in plan for c in row)
    nc = bacc.Bacc("TRN2", target_bir_lowering=False, debug=False,
                   num_devices=N_CORES)

    de_xT = nc.dram_tensor("de_xT", [DIM, S], BF16, kind="ExternalInput")
    en_xT = nc.dram_tensor("en_xT", [DIM, S], BF16, kind="ExternalInput")
    wq = nc.dram_tensor("wq", [DIM, HPC * DH], BF16, kind="ExternalInput")
    wk = nc.dram_tensor("wk", [DIM, HPC * DH], BF16, kind="ExternalInput")
    wv = nc.dram_tensor("wv", [DIM, HPC * DH], BF16, kind="ExternalInput")
    wv_rs = nc.dram_tensor("wv_rs", [128, 2, H * DH], BF16,
                           kind="ExternalInput")
    w1 = nc.dram_tensor("w1", [DIM, DFF], BF16, kind="ExternalInput")
    w2 = nc.dram_tensor("w2", [DFF, DIM], BF16, kind="ExternalInput")
    b1t = nc.dram_tensor("b1t", [128, DFF // 128], F32, kind="ExternalInput")
    b2t = nc.dram_tensor("b2t", [128, DIM // 128], F32, kind="ExternalInput")
    zmask = nc.dram_tensor("zmask", [128, 2 * KC], F32, kind="ExternalInput")
    maskT = None
    if has_mask:
        maskT = nc.dram_tensor("maskT", [S, S], BF16, kind="ExternalInput")
    yT = nc.dram_tensor("yT", [DIM, SL], F32, kind="ExternalOutput")

    with tile.TileContext(nc) as tc:
        _trace(nc, tc, plan, de_xT, en_xT, wq, wk, wv, wv_rs, w1, w2,
               b1t, b2t, zmask, maskT, yT)
    nc.compile()
    return nc, has_mask


def _trace(nc, tc, plan, de_xT, en_xT, wq, wk, wv, wv_rs, w1, w2, b1t, b2t,
           zmask, maskT, yT):
    # Unified allocation stack: everything (pools and single tiles) must be
    # released in strict LIFO order before TileContext exits.
    stack = nc._tile_stack = []   # [release_fn or None(done)]

    def _push(release_fn):
        ent = {"f": release_fn}
        stack.append(ent)
        def rel():
            assert ent["f"] is not None, "double release"
            ent["f"](); ent["f"] = None
        return rel

    def release_rest():
        for ent in reversed(stack):
            if ent["f"] is not None:
                ent["f"](); ent["f"] = None

    noplan = tuple(tuple('N' for _ in range(NB)) for _ in range(NT))

    # ---- pools ----------------------------------------------------------
    def pool(**kw):
        cm = tc.tile_pool(**kw)
        p = cm.__enter__()
        _push(lambda: cm.__exit__(None, None, None))
        return p

    def single(shape, dtype, name):
        t_, f_ = tc.tile(shape, dtype, name=name)
        return t_, _push(f_)

    ps_big = pool(name="ps_big", bufs=2, space="PSUM")
    pv_pool = pool(name="pv", bufs=1, space="PSUM")
    aux_pool = pool(name="aux", bufs=2, space="PSUM")
    pt_pool = pool(name="pt", bufs=6)
    rc_pool = pool(name="rc", bufs=2)
    hn_pool = pool(name="hn", bufs=3)
    q_pool = pool(name="qp", bufs=2)
    k_pool = pool(name="kp", bufs=2)
    v_pool = pool(name="vp", bufs=1)
    sh_pool = pool(name="shp", bufs=2)
    y_pool = pool(name="yp", bufs=2)
    dram = pool(name="dram", bufs=1, space="DRAM")

    # ---- persistent tiles (stack order: frees must pop LIFO) ------------
    wq_sb, _ = single([128, KC, HPC * DH], BF16, "wqsb")
    wk_sb, _ = single([128, KC, HPC * DH], BF16, "wksb")
    wv_sb, _ = single([128, KC, HPC * DH], BF16, "wvsb")
    ident_sb, _ = single([128, 128], BF16, "ident")
    masks.make_identity(nc, ident_sb[:])
    # en allocated BELOW de on the stack: de is freed first (after attn1),
    # en later (after attn2 q/k projections).
    en_sb, en_free = [], []
    for kc in range(KC):
        t_, f_ = single([128, S], BF16, f"en{kc}")
        en_sb.append(t_); en_free.append(f_)
    de_sb, de_free = [], []
    for kc in range(KC):
        t_, f_ = single([128, S], BF16, f"de{kc}")
        de_sb.append(t_); de_free.append(f_)
    # freed right after the attn1 partial-v2 projections (LIFO: above de/en)
    wvrs_sb, wvrs_free = single([128, 2, H * DH], BF16, "wvrssb")
    # DMA trace order: attn1's operands first
    nc.sync.dma_start(wq_sb[:], wq.rearrange("(a p) c -> p a c", p=128))
    nc.sync.dma_start(wk_sb[:], wk.rearrange("(a p) c -> p a c", p=128))
    for kc in range(KC):
        nc.sync.dma_start(de_sb[kc][:], de_xT[kc * 128:(kc + 1) * 128, :])
    nc.sync.dma_start(wv_sb[:], wv.rearrange("(a p) c -> p a c", p=128))
    nc.sync.dma_start(wvrs_sb[:], wv_rs[:])
    for kc in range(KC):
        nc.sync.dma_start(en_sb[kc][:], en_xT[kc * 128:(kc + 1) * 128, :])

    # split collective bounce buffers: partial-v2 ReduceScatter per
    # query-tile half, A2A per head-pair half — each half fires as soon as
    # its producer loop is done, so the wire time hides under the remaining
    # attention compute.
    rs_in = [dram.tile([G * 1024, 256], BF16, name=f"rsi{t}")
             for t in range(NT)]
    rs_out = [dram.tile([1024, 256], BF16, name=f"rso{t}")
              for t in range(NT)]
    cc2_in = [dram.tile([2 * G * 128, SL], BF16, name=f"cc2i{p}")
              for p in range(2)]
    cc2_out = [dram.tile([2 * G * 128, SL], BF16, name=f"cc2o{p}")
               for p in range(2)]

    # ---- helpers --------------------------------------------------------
    def project_qk_pair(x_sb, w_sb, pool, pair):
        """q^T (or k^T) for one head pair as a [128, S] bf16 tile."""
        qt = pool.tile([128, S], BF16, tag="qk", name=f"qk{pair}")
        for st in range(NT):
            ps = ps_big.tile([128, 1024], F32, tag="ps", name="pjps")
            for kc in range(KC):
                for nn in (0, 512):
                    nc.tensor.matmul(
                        ps[:, nn:nn + 512],
                        w_sb[:, kc, pair * 128:(pair + 1) * 128],
                        x_sb[kc][:, st * 1024 + nn:st * 1024 + nn + 512],
                        start=(kc == 0), stop=(kc == KC - 1))
            nc.vector.tensor_copy(qt[:, st * 1024:(st + 1) * 1024], ps[:])
        return qt

    def project_v(src_sb, v_all, blk_lo, blk_hi):
        """v for 4 heads + ones column into v_all[128, NB*HPC*65] (bf16)."""
        for blk in range(blk_lo, blk_hi):
            ps = ps_big.tile([128, 1024], F32, tag="ps", name="vps")
            for kc in range(KC):
                nc.tensor.matmul(
                    ps[:, 0:HPC * DH],
                    src_sb[kc][:, blk * 128:(blk + 1) * 128],
                    wv_sb[:, kc, :],
                    start=(kc == 0), stop=(kc == KC - 1))
            for h in range(HPC):
                nc.vector.tensor_copy(
                    v_all[:, (blk * HPC + h) * 65:(blk * HPC + h) * 65 + 64],
                    ps[:, h * DH:(h + 1) * DH])

    def new_v_all():
        v_all = v_pool.tile([128, NB * HPC * 65], BF16, tag="v", name="vall")
        v3 = v_all[:].rearrange("p (b c) -> p b c", c=65)
        nc.vector.memset(v3[:, :, 64:65], 1.0)
        return v_all

    def attn_unit(q_pairs, k_pairs, v_all, aplan, mask_tiles_in, sh, h, t):
        """scores -> exp -> PV -> normalize for one (head, query-tile).

        PV is computed flipped (stationary = P^T chunk, moving = v||ones), so
        each matmul streams only 65 columns instead of 512 — the PE cost of
        the PV pass drops ~2x. The [s1, e] result is normalized per s1-chunk
        with a per-partition reciprocal, then PE-transposed back to the
        [e, s1] layout the collectives and downstream contractions need.
        """
        pair, off = h // 2, (h % 2) * 64
        pv = [pv_pool.tile([128, 4, 65], F32, tag=f"pv{i}", name=f"pv{i}")
              for i in range(2)]
        # matmul start=True zeroes the WHOLE psum bank, which would clobber
        # the sibling chunks sharing it — pre-zero once and always accumulate.
        for i in range(2):
            nc.vector.memset(pv[i][:], 0.0)
        for blk in range(NB):
            sc = ps_big.tile([128, 1024], F32, tag="ps", name="scps")
            for nn in (0, 512):
                nc.tensor.matmul(
                    sc[:, nn:nn + 512],
                    k_pairs[pair][off:off + 64, blk * 128:(blk + 1) * 128],
                    q_pairs[pair][off:off + 64,
                                  t * 1024 + nn:t * 1024 + nn + 512],
                    start=True, stop=True)
            if aplan[t][blk] == 'M':
                nc.vector.tensor_add(sc[:], sc[:], mask_tiles_in[(t, blk)][:])
            pt = pt_pool.tile([128, 1024], BF16, name="pt")
            nc.scalar.activation(pt[:], sc[:], AF.Exp)
            vs = v_all[:, (blk * HPC + h) * 65:(blk * HPC + h) * 65 + 65]
            for ch in range(8):
                nc.tensor.matmul(
                    pv[ch // 4][:, ch % 4, :],
                    pt[:, ch * 128:(ch + 1) * 128], vs,
                    start=False, stop=(blk == NB - 1),
                    skip_group_check=True)
        rec = rc_pool.tile([128, 8], F32, name="rec")
        for i in range(2):
            nc.vector.reciprocal(rec[:, 4 * i:4 * i + 4], pv[i][:, :, 64])
        for ch in range(8):
            hn = hn_pool.tile([128, 64], BF16, tag="hn", name="hn")
            nc.vector.tensor_scalar_mul(
                hn[:], pv[ch // 4][:, ch % 4, 0:64], rec[:, ch:ch + 1])
            tp = aux_pool.tile([64, 128], BF16, tag="aux", name="tp")
            nc.tensor.transpose(tp[:], hn[:], ident_sb[:])
            nc.vector.tensor_copy(
                sh[pair][off:off + 64,
                         t * 1024 + ch * 128:t * 1024 + (ch + 1) * 128],
                tp[:])

    def new_sh():
        return [sh_pool.tile([128, S], BF16, tag="sh", name=f"sh{p}")
                for p in range(2)]

    # ---- attention 1 (self-attn on de_x, mask) --------------------------
    # pair-0 q/k and v project first so the exp pipeline (ACT) starts as
    # early as possible; pair-1 projections slot in behind the first units.
    q1 = [None, None]
    k1 = [None, None]
    q1[0] = project_qk_pair(de_sb, wq_sb, q_pool, 0)
    k1[0] = project_qk_pair(de_sb, wk_sb, k_pool, 0)
    v1 = new_v_all()
    project_v(de_sb, v1, 0, NB)
    mask_tiles, mask_free = {}, []
    for t in range(NT):
        for blk in range(NB):
            if plan[t][blk] == 'M':
                mt, fm = single([128, 1024], BF16, f"mk{t}_{blk}")
                nc.sync.dma_start(
                    mt[:], maskT[blk * 128:(blk + 1) * 128,
                                 t * 1024:(t + 1) * 1024])
                mask_tiles[(t, blk)] = mt
                mask_free.append(fm)
    # t-major so each query-tile half's partial-v2 ReduceScatter runs while
    # the other half is still computing. Chunk c' of rs_in is the feature
    # block owned by group peer c' — identical slicing on every core, so the
    # SPMD program stays offset-free.
    h1sh = new_sh()
    for t in range(NT):
        for h in range(HPC):
            if t == 0 and h == 2:
                q1[1] = project_qk_pair(de_sb, wq_sb, q_pool, 1)
                k1[1] = project_qk_pair(de_sb, wk_sb, k_pool, 1)
            attn_unit(q1, k1, v1, plan, mask_tiles, h1sh, h, t)
        for sblk in range(NB // NT):
            ps = ps_big.tile([128, 1024], F32, tag="ps", name="rsps")
            col = t * 1024 + sblk * 128
            for kc2 in range(2):
                for nn in (0, 512):
                    nc.tensor.matmul(
                        ps[:, nn:nn + 512],
                        h1sh[kc2][:, col:col + 128],
                        wvrs_sb[:, kc2, nn:nn + 512],
                        start=(kc2 == 0), stop=(kc2 == 1))
            pp = pt_pool.tile([128, 1024], BF16, name="pp")
            nc.vector.tensor_copy(pp[:], ps[:])
            for cp in range(G):
                nc.sync.dma_start(
                    rs_in[t][cp * 1024 + sblk * 128:
                             cp * 1024 + (sblk + 1) * 128, :],
                    pp[:, cp * 256:(cp + 1) * 256])
        nc.gpsimd.collective_compute(
            "ReduceScatter", mybir.AluOpType.add,
            replica_groups=[[0, 1, 2, 3], [4, 5, 6, 7]],
            ins=[rs_in[t].opt()], outs=[rs_out[t].opt()])
    for f in reversed(mask_free):
        f()
    wvrs_free()
    for f in reversed(de_free):
        f()

    # ---- attention 2 (q,k from en_x; v from reduce-scattered v2) --------
    q2 = [None, None]
    k2 = [None, None]
    q2[0] = project_qk_pair(en_sb, wq_sb, q_pool, 0)
    k2[0] = project_qk_pair(en_sb, wk_sb, k_pool, 0)
    q2[1] = project_qk_pair(en_sb, wq_sb, q_pool, 1)
    k2[1] = project_qk_pair(en_sb, wk_sb, k_pool, 1)
    v2 = new_v_all()
    for t in range(NT):
        vrs, vrs_free = single([128, NB // NT, 256], BF16, f"vrs{t}")
        nc.sync.dma_start(vrs[:], rs_out[t].rearrange("(a p) c -> p a c",
                                                      p=128))
        for sblk in range(NB // NT):
            blk = t * (NB // NT) + sblk
            for h in range(HPC):
                nc.vector.tensor_copy(
                    v2[:, (blk * HPC + h) * 65:(blk * HPC + h) * 65 + 64],
                    vrs[:, sblk, h * DH:(h + 1) * DH])
        vrs_free()
    for f in reversed(en_free):
        f()

    # FFN weights prefetch during attention 2
    w1_sb, w1_free = [], []
    for kc in range(KC):
        t_, f_ = single([128, DFF], BF16, f"w1_{kc}")
        nc.sync.dma_start(t_[:], w1[kc * 128:(kc + 1) * 128, :])
        w1_sb.append(t_); w1_free.append(f_)
    w2_sb, w2_free = [], []
    for dc in range(DFF // 128):
        t_, f_ = single([128, DIM], BF16, f"w2_{dc}")
        nc.sync.dma_start(t_[:], w2[dc * 128:(dc + 1) * 128, :])
        w2_sb.append(t_); w2_free.append(f_)
    b1_sb, _ = single([128, DFF // 128], F32, "b1sb")
    b2_sb, _ = single([128, DIM // 128], F32, "b2sb")
    zm_sb, _ = single([128, 2 * KC], F32, "zmsb")
    nc.sync.dma_start(b1_sb[:], b1t[:])
    nc.sync.dma_start(b2_sb[:], b2t[:])
    nc.sync.dma_start(zm_sb[:], zmask[:])

    # pair-major so each head-pair half of h2^T AllToAlls while the other
    # pair is still computing
    h2sh = new_sh()
    for pair in range(2):
        for hh in range(2):
            for t in range(NT):
                attn_unit(q2, k2, v2, noplan, {}, h2sh, pair * 2 + hh, t)
        for j in range(2 * G):
            nc.sync.dma_start(
                cc2_in[pair][j * 128:(j + 1) * 128, :],
                h2sh[pair][:, (j % G) * SL:(j % G + 1) * SL])
        nc.gpsimd.collective_compute(
            "AllToAll", mybir.AluOpType.bypass,
            replica_groups=[[0, 1, 2, 3, 4, 5, 6, 7]],
            ins=[cc2_in[pair].opt()], outs=[cc2_out[pair].opt()])

    # ---- FFN on own S/4 rows -------------------------------------------
    # The A2A delivers each feature row twice (once per batch group); zero the
    # foreign-batch copy via the zmask input, then fold the two copies
    # together so the FFN contraction only runs over the real 1024 rows.
    h2_sb, h2_free = [], []     # [pair][j], j in 0..G
    for pair in range(2):
        row = []
        for i in range(2 * G):
            t_, f_ = single([128, SL], BF16, f"h2_{pair}_{i}")
            nc.sync.dma_start(t_[:], cc2_out[pair][i * 128:(i + 1) * 128, :])
            nc.vector.tensor_scalar_mul(
                t_[:], t_[:], zm_sb[:, pair * 2 * G + i:pair * 2 * G + i + 1])
            row.append(t_); h2_free.append(f_)
        for j in range(G):
            nc.vector.tensor_add(row[j][:], row[j][:], row[j + G][:])
        h2_sb.append(row[:G])

    # pass A: pair-0 partial sums land in SBUF f32 while A2A-b is in flight
    fp_sb, fp_free = [], []
    for dffb in range(DFF // 128):
        ps = ps_big.tile([128, 1024], F32, tag="ps", name="fAps")
        for i in range(G):
            nc.tensor.matmul(
                ps[:, 0:SL],
                w1_sb[2 * i][:, dffb * 128:(dffb + 1) * 128],
                h2_sb[0][i][:],
                start=(i == 0), stop=(i == G - 1))
        t_, f_ = single([128, SL], BF16, f"fp_{dffb}")
        nc.vector.tensor_copy(t_[:], ps[:, 0:SL])
        fp_sb.append(t_); fp_free.append(f_)
    # pass B: pair-1 contribution + pass-A partial, relu, bias
    ff1_sb, ff1_free = [], []
    for dffb in range(DFF // 128):
        ps = ps_big.tile([128, 1024], F32, tag="ps", name="fBps")
        for i in range(G):
            nc.tensor.matmul(
                ps[:, 0:SL],
                w1_sb[2 * i + 1][:, dffb * 128:(dffb + 1) * 128],
                h2_sb[1][i][:],
                start=(i == 0), stop=(i == G - 1))
        nc.vector.tensor_add(ps[:, 0:SL], ps[:, 0:SL], fp_sb[dffb][:])
        t_, f_ = single([128, SL], BF16, f"ff1_{dffb}")
        nc.scalar.activation(t_[:], ps[:, 0:SL], AF.Relu,
                             bias=b1_sb[:, dffb:dffb + 1])
        ff1_sb.append(t_); ff1_free.append(f_)
    for dimb in range(DIM // 128):
        ps = ps_big.tile([128, 1024], F32, tag="ps", name="yps")
        for dc in range(DFF // 128):
            nc.tensor.matmul(
                ps[:, 0:SL],
                w2_sb[dc][:, dimb * 128:(dimb + 1) * 128],
                ff1_sb[dc][:],
                start=(dc == 0), stop=(dc == DFF // 128 - 1))
        ysb = y_pool.tile([128, SL], F32, tag="y", name="ysb")
        nc.vector.tensor_scalar_add(ysb[:], ps[:, 0:SL],
                                    b2_sb[:, dimb:dimb + 1])
        nc.sync.dma_start(yT[dimb * 128:(dimb + 1) * 128, :], ysb[:])

    release_rest()


def _prep_inputs(de_x, en_x, mask, Wq, Wk, Wv, W1, b1, W2, b2, has_mask):
    bf = ml_dtypes.bfloat16
    scale = 1.0 / math.sqrt(DH)
    in_maps = []
    deT = [np.ascontiguousarray(de_x[b].T).astype(bf) for b in range(B)]
    enT = [np.ascontiguousarray(en_x[b].T).astype(bf) for b in range(B)]
    wv_flat = np.transpose(Wv, (1, 0, 2)).reshape(DIM, H * DH)
    w1b = W1.astype(bf)
    w2b = W2.astype(bf)
    b1t = np.ascontiguousarray(b1.reshape(DFF // 128, 128).T).astype(np.float32)
    b2t = np.ascontiguousarray(b2.reshape(DIM // 128, 128).T).astype(np.float32)
    mT = None
    if has_mask:
        mT = np.ascontiguousarray(mask.T * np.float32(NEG)).astype(bf)
    for c in range(N_CORES):
        b, g = divmod(c, G)
        hs = slice(g * HPC, (g + 1) * HPC)
        m = {
            "de_xT": deT[b],
            "en_xT": enT[b],
            "wq": np.ascontiguousarray(
                np.transpose(Wq[hs] * scale, (1, 0, 2)).reshape(DIM, HPC * DH)
            ).astype(bf),
            "wk": np.ascontiguousarray(
                np.transpose(Wk[hs], (1, 0, 2)).reshape(DIM, HPC * DH)).astype(bf),
            "wv": np.ascontiguousarray(
                np.transpose(Wv[hs], (1, 0, 2)).reshape(DIM, HPC * DH)).astype(bf),
            "wv_rs": np.ascontiguousarray(
                wv_flat[g * 256:(g + 1) * 256]
                .reshape(2, 128, H * DH).transpose(1, 0, 2)).astype(bf),
            "w1": w1b, "w2": w2b, "b1t": b1t, "b2t": b2t,
        }
        zm = np.zeros((128, 2 * KC), np.float32)
        for pair in range(2):
            for i in range(2 * G):
                if i // G == b:
                    zm[:, pair * 2 * G + i] = 1.0
        m["zmask"] = zm
        if has_mask:
            m["maskT"] = mT
        in_maps.append(m)
    return in_maps


def get_program(mask):
    plan = _mask_plan(np.asarray(mask))
    if plan not in _CACHE:
        _CACHE[plan] = _build(plan)
    return _CACHE[plan]


_RUNNERS: dict = {}


def _fast_runner(nc):
    """Build (once) a cached jitted SPMD executor for this program.

    run_bass_kernel_spmd re-creates and re-traces its jax.jit closure on
    every call; caching the jitted shard_map shaves seconds of dispatch
    overhead off warm calls. Mirrors bass2jax.run_bass_via_pjrt.
    """
    import jax
    from jax.sharding import Mesh, PartitionSpec
    try:
        from jax.experimental.shard_map import shard_map
    except ImportError:
        from jax.shard_map import shard_map
    import concourse.mybir as _mb
    from concourse import bass2jax as b2j

    b2j.install_neuronx_cc_hook()
    partition_name = (nc.partition_id_tensor.name
                      if nc.partition_id_tensor else None)
    in_names, out_names, out_avals = [], [], []
    for alloc in nc.m.functions[0].allocations:
        if not isinstance(alloc, _mb.MemoryLocationSet):
            continue
        name = alloc.memorylocations[0].name
        if alloc.kind == "ExternalInput":
            if name != partition_name:
                in_names.append(name)
        elif alloc.kind == "ExternalOutput":
            out_names.append(name)
            out_avals.append(jax.core.ShapedArray(
                tuple(alloc.tensor_shape), _mb.dt.np(alloc.dtype)))
    n_params = len(in_names)
    n_outs = len(out_avals)
    all_names = in_names + out_names + ([partition_name] if partition_name else [])
    donate = tuple(range(n_params, n_params + n_outs))

    def _body(*args):
        operands = list(args)
        if partition_name is not None:
            operands.append(b2j.partition_id_tensor())
        return tuple(b2j._bass_exec_p.bind(
            *operands,
            out_avals=tuple(out_avals),
            in_names=tuple(all_names),
            out_names=tuple(out_names),
            lowering_input_output_aliases=(),
            sim_require_finite=True,
            sim_require_nnan=True,
            nc=nc,
        ))

    devices = jax.devices()[:N_CORES]
    mesh = Mesh(np.asarray(devices), ("core",))
    in_specs = (PartitionSpec("core"),) * (n_params + n_outs)
    out_specs = (PartitionSpec("core"),) * n_outs
    sharded = jax.jit(
        shard_map(_body, mesh=mesh, in_specs=in_specs, out_specs=out_specs,
                  check_rep=False),
        donate_argnums=donate, keep_unused=True)

    def runner(in_maps):
        concat_in = [np.concatenate([in_maps[c][nm] for c in range(N_CORES)],
                                    axis=0) for nm in in_names]
        zeros = [np.zeros((N_CORES * a.shape[0], *a.shape[1:]), a.dtype)
                 for a in out_avals]
        out_arrs = sharded(*concat_in, *zeros)
        return [
            {nm: np.asarray(out_arrs[i]).reshape(N_CORES, *out_avals[i].shape)[c]
             for i, nm in enumerate(out_names)}
            for c in range(N_CORES)
        ]

    return runner


def run(inputs, want_results=False, **run_kwargs):
    nc, has_mask = get_program(inputs["mask"])
    in_maps = _prep_inputs(
        inputs["de_x"], inputs["en_x"], inputs["mask"],
        inputs["Wq"], inputs["Wk"], inputs["Wv"],
        inputs["W1"], inputs["b1"], inputs["W2"], inputs["b2"], has_mask)
    results = None
    res = None
    if not run_kwargs:
        try:
            key = id(nc)
            if key not in _RUNNERS:
                _RUNNERS[key] = _fast_runner(nc)
            results = _RUNNERS[key](in_maps)
        except Exception:
            results = None
    if results is None:
        res = bass_utils.run_bass_kernel_spmd(
            nc, in_maps, core_ids=list(range(N_CORES)), **run_kwargs)
        results = res.results
    y = np.empty((B, S, DIM), np.float32)
    for c in range(N_CORES):
        b, g = divmod(c, G)
        y[b, g * SL:(g + 1) * SL, :] = results[c]["yT"].T
    return (y, res) if want_results else y


def kernel(**inputs) -> np.ndarray:
    return run({k: np.asarray(v) for k, v in inputs.items()})

